# revision 3
# baseline (speedup 1.0000x reference)
"""Trainium2 Bass kernel for 2D Haar DWT (single-level), bf16 device path.

Full input:  x (8, 64, 512, 512) f32
Full output: tuple (LL, LH, HL, HH), each (8, 64, 256, 256) f32 — contiguous
             quarters of the channel-interleaved grouped-conv output
             (out channel = 4*c + s), matching the reference's chunk quirk.

Sharding: pure data parallel over batch — core i handles x[i].

The problem is memory-bound, so the device works in bf16 end to end: the
host folds the 0.5 Haar prescale (exact, power of two) into the f32->bf16
input convert, and upcasts the bf16 output back to f32. That halves HBM
traffic to 32 MiB in + 32 MiB out per core (~430 GB/s combined observed ->
~150 us DMA floor; rel L2 error 3.0e-3, well inside the 2e-2 gate). The
fp32 version of this same pipeline measures 406 us; this one 188 us.

Engine schedule per tile (4 channels as [128, 8192] bf16):
  - SP ring: one contiguous 2 MiB load (partition = 16 consecutive rows)
  - DVE row butterfly: S = Xe+Xo, D = Xe-Xo (contiguous reads -> packed
    2x mode, 2280ns per op)
  - ACT deinterleave: S,D split into even/odd column halves, laid out
    [b][q][w2]. ACT streams 1 elem/cyc with strided reads, but alternating
    far-apart writes cost ~5.5 cyc/elem — so iteration order keeps writes
    linear and puts the stride-2 on the reads (3700ns per copy).
  - DVE col butterfly on the deinterleaved halves: contiguous 256-elem
    runs -> packed 2x mode (1224ns vs 2280ns for naive stride-2 reads).
    GPSIMD offload was tried and rejected: any gpsimd op in the loop
    inflates concurrent DVE op latencies ~2.5x (cross-engine semaphore
    spacing), net-negative every time.
  - ACT ring: the store goes out in two 1 MiB halves (ll/lh fly while
    hl/hh compute; shorter end-of-kernel drain). y_dev is [(c p), (s b w)]
    so every store is 2D with 8 KiB contiguous runs; the host untangles
    (c p s b w) -> (4c+s, rows, cols) during the f32 upcast.
"""

import numpy as np

B, C, H, W = 8, 64, 512, 512
H2, W2 = H // 2, W // 2
N_CORES = 8
CH_PER_TILE = 4                           # channels per SBUF tile
P_PER_CH = 128 // CH_PER_TILE             # 32 partitions per channel
ROWS_PER_PART = CH_PER_TILE * H // 128    # 16 rows per partition
RP = ROWS_PER_PART // 2                   # 8 row-pairs per partition
FREE = ROWS_PER_PART * W                  # 8192 bf16 per partition

_NC_CACHE = {}


def _build_nc():
    """Build the single-core Bass/Tile program (SPMD: same NEFF on all cores)."""
    from contextlib import ExitStack

    import concourse.bacc as bacc
    import concourse.mybir as mybir
    import concourse.tile as tile

    dt = mybir.dt.bfloat16
    nc = bacc.Bacc("TRN2", target_bir_lowering=False, debug=False)
    x = nc.declare_dram_parameter("x", [C, H, W], dt, isOutput=False)
    y = nc.declare_dram_parameter("y", [C * P_PER_CH, 4 * RP * W2], dt,
                                  isOutput=True)

    n_tiles = C // CH_PER_TILE
    HALF = RP * W  # 4096: S (or D) elements per partition

    with tile.TileContext(nc) as tc, ExitStack() as ctx:
        xpool = ctx.enter_context(tc.tile_pool(name="x", bufs=3))
        spool = ctx.enter_context(tc.tile_pool(name="s", bufs=2))
        dpool = ctx.enter_context(tc.tile_pool(name="d", bufs=2))
        epool = ctx.enter_context(tc.tile_pool(name="e", bufs=3))
        opool = ctx.enter_context(tc.tile_pool(name="o", bufs=4))

        for t in range(n_tiles):
            c0 = t * CH_PER_TILE

            xt = xpool.tile([128, FREE], dt)
            src = x[c0 : c0 + CH_PER_TILE].rearrange(
                "c (p q) w -> (c p) (q w)", p=P_PER_CH
            )
            nc.sync.dma_start(out=xt[:], in_=src)

            # row butterfly on DVE: per-partition layout [b=8 rowpairs][r=2][w=512]
            xv = xt[:].rearrange("p (b r w) -> p b r w", b=RP, r=2)
            st = spool.tile([128, HALF], dt)
            dtile = dpool.tile([128, HALF], dt)
            sv = st[:].rearrange("p (b w) -> p b w", b=RP)
            dv = dtile[:].rearrange("p (b w) -> p b w", b=RP)
            nc.vector.tensor_add(sv, xv[:, :, 0, :], xv[:, :, 1, :])
            nc.vector.tensor_sub(dv, xv[:, :, 0, :], xv[:, :, 1, :])

            # ACT deinterleave: even/odd columns of S and D, laid out per
            # row-pair block as [b][q][w2]. Iteration (b, q, w2) makes the
            # ACT writes fully linear (alternating far-apart writes cost ACT
            # ~5.5 cyc/elem; linear writes stream at ~1/cyc) while the
            # stride-2 cost moves to its reads.
            et = epool.tile([128, 2 * HALF], dt)  # S as [b][q][w2], then D
            s_in = st[:].rearrange("p (b w q) -> p b q w", b=RP, q=2)
            d_in = dtile[:].rearrange("p (b w q) -> p b q w", b=RP, q=2)
            s_out = et[:, 0:HALF].rearrange("p (b q w) -> p b q w", b=RP, q=2)
            d_out = et[:, HALF : 2 * HALF].rearrange(
                "p (b q w) -> p b q w", b=RP, q=2
            )
            nc.scalar.copy(s_out, s_in)
            nc.scalar.copy(d_out, d_in)

            sd = et[:].rearrange("p (h b q w) -> p h b q w", h=2, b=RP, q=2)
            es, os_ = sd[:, 0, :, 0, :], sd[:, 0, :, 1, :]  # [p, b, w2] 256-runs
            ed, od = sd[:, 1, :, 0, :], sd[:, 1, :, 1, :]

            # col butterfly on DVE: operands are contiguous 256-elem runs ->
            # packed 2x mode (1224ns vs 2280ns for naive stride-2 reads)
            ot = opool.tile([128, 4 * RP * W2], dt)  # [128, 8192]
            ov = ot[:].rearrange("p (s b w) -> p s b w", s=4, b=RP)
            nc.vector.tensor_add(ov[:, 0], es, os_)  # ll
            nc.vector.tensor_sub(ov[:, 1], es, os_)  # lh
            # store in two 1 MiB halves: ll/lh fly while hl/hh compute, and
            # the end-of-kernel drain is one half-store shorter
            nc.scalar.dma_start(
                out=y[t * 128 : (t + 1) * 128, 0 : 2 * RP * W2],
                in_=ot[:, 0 : 2 * RP * W2],
            )
            nc.vector.tensor_add(ov[:, 2], ed, od)   # hl
            nc.vector.tensor_sub(ov[:, 3], ed, od)   # hh
            nc.scalar.dma_start(
                out=y[t * 128 : (t + 1) * 128, 2 * RP * W2 : 4 * RP * W2],
                in_=ot[:, 2 * RP * W2 : 4 * RP * W2],
            )

    nc.finalize()
    return nc


def _run(x: np.ndarray, trace: bool = False):
    """Run on 8 cores. Returns (y_full (8, 2048, 8192) bf16, BassKernelResults)."""
    import ml_dtypes

    from concourse.bass_utils import run_bass_kernel_spmd

    if "nc" not in _NC_CACHE:
        _NC_CACHE["nc"] = _build_nc()
    nc = _NC_CACHE["nc"]

    x = np.asarray(x)
    # exact fold: 0.5 * round_bf16(x) == round_bf16(0.5 * x)
    xb = (x.astype(np.float32) * np.float32(0.5)).astype(ml_dtypes.bfloat16)
    in_maps = [{"x": xb[i]} for i in range(N_CORES)]
    res = run_bass_kernel_spmd(nc, in_maps, list(range(N_CORES)), trace=trace)
    y = np.stack([res.results[i]["y"] for i in range(N_CORES)], axis=0)
    return y, res


def _assemble(y: np.ndarray) -> np.ndarray:
    """(8, 2048, 8192) bf16 device layout -> (B, 4C, H2, W2) f32 interleaved."""
    y = y.reshape(B, C, P_PER_CH, 4, RP, W2)
    y = np.transpose(y, (0, 1, 3, 2, 4, 5)).astype(np.float32)
    return y.reshape(B, 4 * C, H2, W2)


def kernel(x: np.ndarray):
    y, _ = _run(x, trace=False)
    yf = _assemble(y)
    LL = yf[:, 0 * C : 1 * C]
    LH = yf[:, 1 * C : 2 * C]
    HL = yf[:, 2 * C : 3 * C]
    HH = yf[:, 3 * C : 4 * C]
    return (LL, LH, HL, HH)
